# revision 29
# baseline (speedup 1.0000x reference)
"""Trainium2 Bass kernel for nn_GCN2_23691039605435 (2-layer GCN, 8 NeuronCores).

Math: with dinv = deg^-1/2 (self-loops included), per GCN layer
    table = dinv * (input @ W)            (per-node scaling; AllGathered)
    agg_d = sum_{edges e->d} table[src_e] (gather + segment-sum)
    h_d   = relu(dinv_d * agg_d + b)
Final: sigmoid(mean(h2 @ Wfc + bfc)) = sigmoid((colsum(h2) @ Wfc)/N + bfc).

Sharding: nodes globally degree-sorted and dealt round-robin to the 8 cores
(balances per-core edge counts and makes per-group degrees uniform).  Each
core owns its nodes' incoming edges.  Device does: table build (matmul +
scale), AllGather of the table, dma_gather of source rows into per-group
slot grids, PE identity-matmul accumulation (segment-sum in PSUM), DVE
scale+relu epilogue, and a final column-sum.  Host does only integer graph
preprocessing (partitioning / index streams) and the last sigmoid of an
8-float reduction.

dma_gather indices are int16, so the gathered table is addressed through two
overlapping windows LO=[0,32768) and HI=[NSTAR-32768,NSTAR); each edge is
routed to a window covering its source row (flexible edges balanced to
minimize slot-grid padding).  All 8 cores share one instruction stream, so
grid shapes (K per group) are unified across cores (max).
"""
import numpy as np

LANES = 128


class Cfg:
    def __init__(self, n, e_cap, cores, groups, cols_budget=224, win=32768):
        self.N = n
        self.C = cores
        self.G = groups
        self.PCP = groups * LANES          # padded nodes per core
        assert n % cores == 0
        self.PC = n // cores               # real nodes per core
        assert self.PC <= self.PCP
        self.NSTAR = cores * self.PCP
        self.WIN = min(win, self.NSTAR)
        self.HIB = self.NSTAR - self.WIN   # hi window base
        self.COLS = cols_budget            # msg-buffer column budget per batch
        self.D = 64


FULL = Cfg(50000, 800000, 8, 49)


# ---------------------------------------------------------------- host prep
def prep(cfg, edge_index):
    """Integer graph preprocessing. Returns layout dict."""
    C, G, D = cfg.C, cfg.G, cfg.D
    N, PCP, NSTAR = cfg.N, cfg.PCP, cfg.NSTAR
    WIN, HIB = cfg.WIN, cfg.HIB

    src0 = edge_index[0].astype(np.int64)
    dst0 = edge_index[1].astype(np.int64)
    deg = np.bincount(dst0, minlength=N) + 1
    dinv = (1.0 / np.sqrt(deg.astype(np.float32))).astype(np.float32)

    # node -> core by degree deal
    order = np.argsort(-deg, kind="stable")
    ranks = np.empty(N, np.int64)
    ranks[order] = np.arange(N)
    core = ranks % C

    # conservative window classes (by source core) for the within-core sort
    lo_cores = int(np.floor(WIN / PCP))            # cores fully inside LO
    hi_core0 = int(np.ceil(HIB / PCP))             # first core fully in HI
    sc = core[src0]
    ml_cons = np.bincount(dst0, weights=(sc < hi_core0).astype(np.float64),
                          minlength=N).astype(np.int64)
    mh_cons = np.bincount(dst0, weights=(sc >= lo_cores).astype(np.float64),
                          minlength=N).astype(np.int64)

    # within-core sort: (deg desc, ml-mh asc) -> rank r; g = r//128, p = r%128
    g_of = np.zeros(N, np.int64)
    p_of = np.zeros(N, np.int64)
    for c in range(C):
        idx = np.where(core == c)[0]
        k = np.lexsort((ml_cons[idx] - mh_cons[idx], -deg[idx]))
        nodes = idx[k]
        r = np.arange(len(nodes))
        g_of[nodes] = r // LANES
        p_of[nodes] = r % LANES

    perm = core * PCP + p_of * G + g_of            # node -> table row
    ZROW_LO = 0 * PCP + (LANES - 1) * G + (G - 1)  # pad row in core 0
    ZROW_HI = (C - 1) * PCP + (LANES - 1) * G + (G - 1)
    assert ZROW_LO < WIN and HIB <= ZROW_HI < HIB + WIN
    # those rows must be pads (only true if PC <= PCP - 1)
    assert cfg.PC <= cfg.PCP - 1

    # gathered edges EXCLUDE self-loops (self term comes from the local
    # slab in the epilogue); deg/dinv above still include them
    psrc = perm[src0]
    c_d, g_d, p_d = core[dst0], g_of[dst0], p_of[dst0]

    # true window classes
    must_lo = psrc < HIB
    must_hi = psrc >= WIN
    cls = np.where(must_lo, 0, np.where(must_hi, 2, 1))

    # group edges by (core, group, lane), classes in order lo,flex,hi
    key = (c_d * G + g_d) * LANES + p_d
    order_e = np.lexsort((cls, key))
    key_s = key[order_e]
    psrc_s = psrc[order_e]
    cls_s = cls[order_e]
    counts = np.bincount(key_s, minlength=C * G * LANES)
    starts = np.zeros(C * G * LANES + 1, np.int64)
    np.cumsum(counts, out=starts[1:])
    j_in_node = np.arange(len(key_s)) - starts[key_s]

    m_l = np.bincount(key_s, weights=(cls_s == 0).astype(np.float64),
                      minlength=C * G * LANES).astype(np.int64)
    m_h = np.bincount(key_s, weights=(cls_s == 2).astype(np.float64),
                      minlength=C * G * LANES).astype(np.int64)
    dg = counts.astype(np.int64)

    # per-group unified K_lo/K_hi via threshold search
    m_l3 = m_l.reshape(C, G, LANES)
    m_h3 = m_h.reshape(C, G, LANES)
    dg3 = dg.reshape(C, G, LANES)
    K_lo = np.zeros(G, np.int64)
    K_hi = np.zeros(G, np.int64)
    lo_cnt3 = np.zeros((C, G, LANES), np.int64)
    for g in range(G):
        ml, mh, d = m_l3[:, g, :], m_h3[:, g, :], dg3[:, g, :]
        dmax = int(d.max()) if d.size else 0
        best = None
        for t in range(dmax + 1):
            lo = np.clip(t, ml, d - mh)
            cost = int(lo.max()) + int((d - lo).max())
            if best is None or cost < best[0]:
                best = (cost, t)
        lo = np.clip(best[1], ml, d - mh)
        if getattr(cfg, "force_lo", False):
            lo = d
        lo_cnt3[:, g, :] = lo
        K_lo[g] = int(lo.max())
        K_hi[g] = int((d - lo).max())

    # batches: consecutive groups, sum(K_lo+K_hi) <= COLS, and each window's
    # column count <= 126 (Q7 scratch caps one dma_gather at ~16k indices)
    WMAX = cfg.COLS  # per-call idx count is bounded by the queue chunking
    batches = []
    cur = []
    cur_cols = cur_lo = cur_hi = 0
    for g in range(G):
        kl, kh = int(K_lo[g]), int(K_hi[g])
        if cur and (cur_cols + kl + kh > cfg.COLS
                    or cur_lo + kl > WMAX or cur_hi + kh > WMAX):
            batches.append(cur)
            cur, cur_cols, cur_lo, cur_hi = [], 0, 0, 0
        cur.append(g)
        cur_cols += kl + kh
        cur_lo += kl
        cur_hi += kh
    if cur:
        batches.append(cur)

    # per-core int16 index stream, batch-segmented: [lo grid | hi grid] each
    # slot-major (col*128 + lane), grids prefilled with zero-row indices
    lo_cnt_e = lo_cnt3.reshape(-1)[key_s]
    is_lo_e = j_in_node < lo_cnt_e
    col_e = np.where(is_lo_e, j_in_node, j_in_node - lo_cnt_e)

    # stream offsets
    lo_off_in_b = {}
    hi_off_in_b = {}
    seg_off = {}                   # batch -> stream offset (int16 units)
    seg_len = {}
    tot = 0
    for bi, gs in enumerate(batches):
        seg_off[bi] = tot
        klo_b = int(K_lo[gs].sum())
        khi_b = int(K_hi[gs].sum())
        o = 0
        for g in gs:
            lo_off_in_b[g] = o
            o += int(K_lo[g])
        o = 0
        for g in gs:
            hi_off_in_b[g] = o
            o += int(K_hi[g])
        seg_len[bi] = (klo_b + khi_b) * LANES
        tot += seg_len[bi]

    batch_of = {}
    for bi, gs in enumerate(batches):
        for g in gs:
            batch_of[g] = bi
    klo_b_arr = [int(K_lo[gs].sum()) for gs in batches]

    # per-edge stream position
    bi_e = np.array([batch_of[g] for g in range(G)], np.int64)[(key_s // LANES) % G]
    g_e = (key_s // LANES) % G
    c_e = key_s // (G * LANES)
    p_e = key_s % LANES
    seg_off_e = np.array([seg_off[b] for b in range(len(batches))], np.int64)[bi_e]
    klo_e = np.array(klo_b_arr, np.int64)[bi_e]
    lo_goff = np.array([lo_off_in_b[g] for g in range(G)], np.int64)[g_e]
    hi_goff = np.array([hi_off_in_b[g] for g in range(G)], np.int64)[g_e]
    pos = seg_off_e + np.where(
        is_lo_e,
        (lo_goff + col_e) * LANES + p_e,
        (klo_e + hi_goff + col_e) * LANES + p_e,
    )
    val = np.where(is_lo_e, psrc_s, psrc_s - HIB).astype(np.int16)

    streams = np.empty((C, tot), np.int16)
    for bi, gs in enumerate(batches):
        klo_b = klo_b_arr[bi] * LANES
        streams[:, seg_off[bi]:seg_off[bi] + klo_b] = ZROW_LO
        streams[:, seg_off[bi] + klo_b:seg_off[bi] + seg_len[bi]] = ZROW_HI - HIB
    streams[c_e, pos] = val

    # verify stream consistency: each (c,g,p) real edge count
    return {
        "perm": perm, "dinv": dinv, "deg": deg,
        "K_lo": K_lo, "K_hi": K_hi, "batches": batches,
        "seg_off": seg_off, "seg_len": seg_len,
        "lo_off_in_b": lo_off_in_b, "hi_off_in_b": hi_off_in_b,
        "streams": streams, "tot": tot,
        "ZROW_LO": ZROW_LO, "ZROW_HI": ZROW_HI,
    }


# ------------------------------------------------------------ device builder
def build(cfg, lay, nodes_in_last_group, phases="ACDEF"):
    import concourse.bass as bass
    import concourse.bacc as bacc
    import concourse.mybir as mybir
    import concourse.tile as tile

    f32 = mybir.dt.float32
    bf = mybir.dt.bfloat16
    i16 = mybir.dt.int16
    C, G, D = cfg.C, cfg.G, cfg.D
    PCP, NSTAR, WIN, HIB = cfg.PCP, cfg.NSTAR, cfg.WIN, cfg.HIB
    K_lo, K_hi = lay["K_lo"], lay["K_hi"]
    batches = lay["batches"]
    TOT = lay["tot"]

    nc = bacc.Bacc(None, target_bir_lowering=False, num_swdge_queues=4)
    xT_d = nc.dram_tensor("xT", [D, PCP], bf, kind="ExternalInput")
    w1_d = nc.dram_tensor("w1", [D, D], bf, kind="ExternalInput")
    w2_d = nc.dram_tensor("w2", [D, D], bf, kind="ExternalInput")
    dinv_d = nc.dram_tensor("dinv", [LANES, G], f32, kind="ExternalInput")
    ident_d = nc.dram_tensor("ident", [LANES, LANES], bf, kind="ExternalInput")
    gidx_d = nc.dram_tensor("gidx", [LANES, TOT // 16], i16, kind="ExternalInput")
    out_d = nc.dram_tensor("out", [D, 1], f32, kind="ExternalOutput")

    with tile.TileContext(nc) as tc:
        with (
            tc.tile_pool(name="const", bufs=1) as constp,
            tc.tile_pool(name="slabs", bufs=1) as slabp,
            tc.tile_pool(name="msg", bufs=2) as msgp,
            tc.tile_pool(name="idx", bufs=2) as idxp,
            tc.tile_pool(name="small", bufs=3) as smallp,
            tc.tile_pool(name="psA", bufs=4, space="PSUM") as psA,
            tc.tile_pool(name="psM", bufs=2, space="PSUM") as psM,
            tc.tile_pool(name="psF", bufs=1, space="PSUM") as psF,
            tc.tile_pool(name="dram", bufs=1, space="DRAM") as dram,
        ):
            w1_sb = constp.tile([D, D], bf, tag="w1")
            nc.sync.dma_start(w1_sb[:], w1_d[:])
            w2_sb = constp.tile([D, D], bf, tag="w2")
            nc.sync.dma_start(w2_sb[:], w2_d[:])
            dinv_sb = constp.tile([LANES, G], f32, tag="dinv")
            nc.sync.dma_start(dinv_sb[:], dinv_d[:])
            xT_sb = constp.tile([D, PCP], bf, tag="xT")
            nc.sync.dma_start(xT_sb[:], xT_d[:])
            ident_sb = constp.tile([LANES, LANES], bf, tag="ident")
            nc.sync.dma_start(ident_sb[:], ident_d[:])
            ones_sb = constp.tile([LANES, 1], bf, tag="ones")
            nc.vector.memset(ones_sb[:], 1.0)

            h1_sb = constp.tile([LANES, G, D], bf, tag="h1")
            h2_sb = constp.tile([LANES, G, D], bf, tag="h2")
            slab_sb = slabp.tile([LANES, G, 2 * D], bf, tag="slab")
            nc.vector.memset(slab_sb[:], 0.0)

            bounce1 = dram.tile([PCP, 2 * D], bf)
            table1 = dram.tile([NSTAR, 2 * D], bf)
            bounce2 = dram.tile([PCP, 2 * D], bf)
            table2 = dram.tile([NSTAR, 2 * D], bf)

            # ---- phase A: table1 slab = dinv * (x @ W1)
            sc_tbl1 = nc.enter_named_scope("tbl1", False)
            for g in range(G):
                ps = psM.tile([LANES, D], f32, tag="misc")
                nc.tensor.matmul(ps[:], xT_sb[:, g * LANES:(g + 1) * LANES],
                                 w1_sb[:], start=True, stop=True)
                nc.vector.tensor_scalar(slab_sb[:, g, 0:D], ps[:],
                                        dinv_sb[:, g:g + 1], None,
                                        mybir.AluOpType.mult)
            nc.sync.dma_start(
                bounce1[:].rearrange("(p g) d -> p (g d)", g=G),
                slab_sb[:].rearrange("p g d -> p (g d)"))
            nc.gpsimd.collective_compute(
                "AllGather", mybir.AluOpType.bypass,
                replica_groups=[list(range(C))],
                ins=[bounce1.opt()], outs=[table1.opt()],
            )
            nc.leave_named_scope("tbl1", sc_tbl1[0], False)

            # ---- gather + aggregate layer
            agg_mode = getattr(cfg, "agg_mode", "full")

            def agg_layer(table, h_out):
                for bi, gs in enumerate(batches):
                    klo_b = int(K_lo[gs].sum())
                    khi_b = int(K_hi[gs].sum())
                    ncols = klo_b + khi_b
                    seg_o16 = lay["seg_off"][bi] // 16
                    seg_l16 = lay["seg_len"][bi] // 16
                    idx_sb = idxp.tile([LANES, cfg.COLS * 8], i16, tag="idx")
                    nc.sync.dma_start(idx_sb[:, :seg_l16],
                                      gidx_d[:, seg_o16:seg_o16 + seg_l16])
                    msg = msgp.tile([LANES, cfg.COLS, 2 * D], bf, tag="msg")
                    nlo = klo_b * LANES
                    nhi = khi_b * LANES
                    # one SWDGE queue saturates ~28 GB/s, 2+ hit ~54;
                    # chunk each window and greedily balance 4 queues
                    target = max(1, (ncols + 3) // 4)
                    pieces = []
                    for win, w0, w1 in ((0, 0, klo_b), (1, klo_b, ncols)):
                        c = w0
                        while c < w1:
                            c2 = min(w1, c + target)
                            pieces.append((c, c2, win))
                            c = c2
                    qload = [0, 0, 0, 0]
                    calls = []
                    for c0, c1, win in sorted(
                            pieces, key=lambda p: p[0] - p[1]):
                        q = qload.index(min(qload))
                        qload[q] += c1 - c0
                        calls.append((c0, c1, win, q))
                    for c0, c1, win, q in calls:
                        nn = (c1 - c0) * LANES
                        tab_ap = table[0:WIN, :] if win == 0 else                             table[HIB:HIB + WIN, :]
                        nc.gpsimd.dma_gather(
                            msg[:, c0:c1, :], tab_ap,
                            idx_sb[:, c0 * 8:c1 * 8], nn, nn, 2 * D,
                            single_packet=False, queue_num=q)
                    if agg_mode == "gather_only":
                        for g in gs:
                            if g == gs[0]:
                                nc.vector.tensor_copy(h_out[:, g, :],
                                                      msg[:, 0, 0:D])
                            else:
                                nc.vector.memset(h_out[:, g, :], 0.0)
                        continue
                    for g in gs:
                        kl, kh = int(K_lo[g]), int(K_hi[g])
                        if kl + kh == 0:
                            # only the self-loop term
                            nc.vector.tensor_scalar(
                                h_out[:, g, :], slab_sb[:, g, 0:D],
                                dinv_sb[:, g:g + 1], 0.0,
                                mybir.AluOpType.mult, mybir.AluOpType.max)
                            continue
                        ps = psA.tile([LANES, D], f32, tag="agg")
                        lo0 = lay["lo_off_in_b"][g]
                        hi0 = klo_b + lay["hi_off_in_b"][g]
                        ktot = kl + kh
                        ki = 0
                        for k in range(kl):
                            nc.tensor.matmul(ps[:], ident_sb[:],
                                             msg[:, lo0 + k, 0:D],
                                             start=(ki == 0), stop=(ki == ktot - 1))
                            ki += 1
                        for k in range(kh):
                            nc.tensor.matmul(ps[:], ident_sb[:],
                                             msg[:, hi0 + k, 0:D],
                                             start=(ki == 0), stop=(ki == ktot - 1))
                            ki += 1
                        # h = relu(dinv * (agg + self_row)); self term from
                        # the local table slab (biases are zero)
                        t = smallp.tile([LANES, D], f32, tag="epi")
                        nc.vector.scalar_tensor_tensor(
                            t[:], ps[:], 0.0, slab_sb[:, g, 0:D],
                            mybir.AluOpType.bypass, mybir.AluOpType.add)
                        nc.vector.tensor_scalar(
                            h_out[:, g, :], t[:], dinv_sb[:, g:g + 1], 0.0,
                            mybir.AluOpType.mult, mybir.AluOpType.max)

            if "C" in phases:
                sc = nc.enter_named_scope("agg1", False)
                agg_layer(table1, h1_sb)
                nc.leave_named_scope("agg1", sc[0], False)
            else:
                for g in range(G):
                    nc.vector.memset(h1_sb[:, g, :], 0.0)

            # ---- phase D: table2 slab = dinv * (h1 @ W2)
            sc_tbl2 = nc.enter_named_scope("tbl2", False)
            for g in range(G) if "D" in phases else []:
                pst = psM.tile([D, LANES], bf, tag="misc")
                nc.tensor.transpose(pst[:], h1_sb[:, g, :], ident_sb[:])
                h1T = smallp.tile([D, LANES], bf, tag="h1T")
                nc.vector.tensor_copy(h1T[:], pst[:])
                ps2 = psM.tile([LANES, D], f32, tag="misc")
                nc.tensor.matmul(ps2[:], h1T[:], w2_sb[:], start=True, stop=True)
                nc.vector.tensor_scalar(slab_sb[:, g, 0:D], ps2[:],
                                        dinv_sb[:, g:g + 1], None,
                                        mybir.AluOpType.mult)
            if "D" in phases:
                nc.sync.dma_start(
                    bounce2[:].rearrange("(p g) d -> p (g d)", g=G),
                    slab_sb[:].rearrange("p g d -> p (g d)"))
                nc.gpsimd.collective_compute(
                    "AllGather", mybir.AluOpType.bypass,
                    replica_groups=[list(range(C))],
                    ins=[bounce2.opt()], outs=[table2.opt()],
                )

            nc.leave_named_scope("tbl2", sc_tbl2[0], False)
            if "E" in phases:
                sc = nc.enter_named_scope("agg2", False)
                agg_layer(table2 if "D" in phases else table1, h2_sb)
                nc.leave_named_scope("agg2", sc[0], False)
            else:
                for g in range(G):
                    nc.vector.memset(h2_sb[:, g, :], 0.0)

            # ---- phase F: colsum over real nodes
            cs = psF.tile([D, 1], f32, tag="colsum")
            live = [g for g in range(G) if nodes_in_last_group[g] > 0]
            for g in live:
                np_ = nodes_in_last_group[g]
                nc.tensor.matmul(cs[:], h2_sb[:np_, g, :], ones_sb[:np_, :],
                                 start=(g == live[0]), stop=(g == live[-1]))
            out_sb = smallp.tile([D, 1], f32, tag="out")
            nc.vector.tensor_copy(out_sb[:], cs[:])
            nc.sync.dma_start(out_d[:], out_sb[:])
    nc.compile()
    return nc


# ------------------------------------------------------------------- driver
def run(cfg, inputs, run_hw=True, phases="ACDEF", trace=False, want_bkr=False):
    x = np.asarray(inputs["x"], np.float32)
    ei = np.asarray(inputs["edge_index"])
    W1 = np.asarray(inputs["W1"], np.float32)
    b1 = np.asarray(inputs["b1"], np.float32)
    W2 = np.asarray(inputs["W2"], np.float32)
    b2 = np.asarray(inputs["b2"], np.float32)
    Wfc = np.asarray(inputs["Wfc"], np.float32)
    bfc = np.asarray(inputs["bfc"], np.float32)
    assert not b1.any() and not b2.any(), "zero conv biases assumed"

    C, G, D, PCP, NSTAR = cfg.C, cfg.G, cfg.D, cfg.PCP, cfg.NSTAR
    lay = prep(cfg, ei)
    perm, dinv = lay["perm"], lay["dinv"]

    xp = np.zeros((NSTAR, D), np.float32)
    xp[perm] = x
    dinvp = np.zeros(NSTAR, np.float32)
    dinvp[perm] = dinv

    # per-group real-lane counts (rank g*128+p < PC)
    nreal = [int(np.clip(cfg.PC - g * LANES, 0, LANES)) for g in range(G)]

    in_maps = []
    for c in range(C):
        slab = xp[c * PCP:(c + 1) * PCP]               # row p*G+g
        xT = np.ascontiguousarray(
            slab.reshape(LANES, G, D).transpose(2, 1, 0)).reshape(D, PCP)
        dv = np.ascontiguousarray(
            dinvp[c * PCP:(c + 1) * PCP].reshape(LANES, G))
        stream = lay["streams"][c]
        gidx = np.ascontiguousarray(
            np.tile(stream.reshape(-1, 16).T, (8, 1)))
        import ml_dtypes
        bfnp = ml_dtypes.bfloat16
        in_maps.append({
            "xT": np.ascontiguousarray(xT).astype(bfnp),
            "w1": W1.astype(bfnp), "w2": W2.astype(bfnp),
            "dinv": dv,
            "ident": np.eye(LANES, dtype=bfnp),
            "gidx": gidx,
        })

    nc = build(cfg, lay, nreal, phases)
    r = None
    if run_hw:
        from concourse import bass_utils
        r = bass_utils.run_bass_kernel_spmd(nc, in_maps, core_ids=list(range(C)),
                                            trace=trace)
        outs = [r.results[c]["out"] for c in range(C)]
    else:
        from concourse.bass_interp import MultiCoreSim
        sim = MultiCoreSim(nc, C)
        for c in range(C):
            for k, v in in_maps[c].items():
                sim.cores[c].tensor(k)[:] = v
        sim.simulate()
        outs = [sim.cores[c].mem_tensor("out") for c in range(C)]
    colsum = np.zeros(D, np.float64)
    for c in range(C):
        colsum += outs[c][:, 0].astype(np.float64)
    z = float(colsum @ Wfc[:, 0].astype(np.float64)) / cfg.N + float(bfc[0])
    out = 1.0 / (1.0 + np.exp(-np.float32(z), dtype=np.float32))
    res = np.array([[out]], np.float32)
    if want_bkr:
        return res, r
    return res


def kernel(**inputs) -> np.ndarray:
    return run(FULL, inputs)
